# revision 1
# baseline (speedup 1.0000x reference)
"""Single-head causal attention (B=8, T=2048, C=1024, H=64) on 8 TRN2 NeuronCores.

Sharding: data-parallel over batch -- core b computes batch element b. No
collectives. Per core the kernel computes, for x_b [T, C]:
    q = x_b @ Wq / sqrt(H); k = x_b @ Wk; v = x_b @ Wv
    out = softmax(causal(q @ k.T)) @ v

Layout strategy (all matmuls in float32r, 1 cycle/row at N>=256):
  - Host passes xT = x_b.T [C, T] so the contraction dim C lands on SBUF
    partitions without any on-device transpose of x.
  - Projections produce Q^T/K^T stacked [128, T] (one matmul per (c,n) chunk)
    and V^T [64, T]; V^T is re-tiled to natural V [128k, 64h] tiles with DVE
    32x32 block transposes, with a ones-column appended (softmax denominators
    fall out of the O matmul for free).
  - Attention per 512-wide q-group g: S^T[k,q] tiles = K^T_j.T @ Q^T_g; P^T =
    exp(S^T) on ACT (no max subtraction -- scores are bounded ~ +-17 here, and
    exp stays well inside fp32 range); causal masking is a 0/1 multiply on the
    single diagonal 128x128 sub-block; O^T[65, 512] += Vones_j.T @ P^T
    accumulates over k-tiles in PSUM. Row 64 of O^T is the softmax denominator.
  - Normalize: DVE reciprocal of the denominator row, GPSIMD partition
    broadcast, DVE multiply; DMA O^T [64, T] out. Host transposes back.
"""

from contextlib import ExitStack

import numpy as np

import concourse.mybir as mybir
import concourse.tile as tile
from concourse import bacc
from concourse.bass_utils import run_bass_kernel_spmd
from concourse.masks import make_identity, make_upper_triangular

B, T, C, H = 8, 2048, 1024, 64
N_CORES = 8
GQ = 512          # q-group width (PSUM bank)
NG = T // GQ      # 4 q-groups
KT = 128          # k-tile size
CC = C // 128     # 8 contraction chunks
F32 = mybir.dt.float32
F32R = mybir.dt.float32r
EXP = mybir.ActivationFunctionType.Exp


def _emit(ctx, tc):
    nc = tc.nc
    xT = nc.dram_tensor("xT", [C, T], F32R, kind="ExternalInput").ap()
    wqk = nc.dram_tensor("wqk", [C, 2 * H], F32R, kind="ExternalInput").ap()
    wv = nc.dram_tensor("wv", [C, H], F32R, kind="ExternalInput").ap()
    outT = nc.dram_tensor("outT", [H, T], F32, kind="ExternalOutput").ap()

    const = ctx.enter_context(tc.tile_pool(name="const", bufs=1))
    persist = ctx.enter_context(tc.tile_pool(name="persist", bufs=1))
    xt_pool = ctx.enter_context(tc.tile_pool(name="xt", bufs=10))
    pt_pool = ctx.enter_context(tc.tile_pool(name="pt", bufs=4))
    out_pool = ctx.enter_context(tc.tile_pool(name="outp", bufs=3))
    div_pool = ctx.enter_context(tc.tile_pool(name="div", bufs=2))
    ps_qk = ctx.enter_context(tc.tile_pool(name="ps_qk", bufs=2, space="PSUM"))
    ps_v = ctx.enter_context(tc.tile_pool(name="ps_v", bufs=1, space="PSUM"))
    ps_s = ctx.enter_context(tc.tile_pool(name="ps_s", bufs=2, space="PSUM"))
    ps_o = ctx.enter_context(tc.tile_pool(name="ps_o", bufs=2, space="PSUM"))
    ps_tr = ctx.enter_context(tc.tile_pool(name="ps_tr", bufs=1, space="PSUM"))

    wqk_sb = const.tile([128, CC, 2 * H], F32R)
    nc.sync.dma_start(out=wqk_sb[:], in_=wqk.rearrange("(c p) m -> p c m", p=128))
    wv_sb = const.tile([128, CC, H], F32R)
    nc.sync.dma_start(out=wv_sb[:], in_=wv.rearrange("(c p) m -> p c m", p=128))
    # mask[p, f] = 1.0 where p <= f else 0 : keep k_local <= q_local.
    mask = const.tile([128, 128], F32)
    make_upper_triangular(nc, mask[:], val=1.0, diag=True)
    # f32r identity for PE-transpose of V^T, staged through an f32 tile since
    # memset/affine_select can't emit f32r but ACT can round into it.
    ident_f = const.tile([H, H], F32)
    make_identity(nc, ident_f[:])
    ident = const.tile([H, H], F32R)
    nc.scalar.copy(ident[:], ident_f[:])

    qt = persist.tile([H, T], F32R)             # Q^T (pre-scaled by 1/sqrt(H))
    kt = persist.tile([H, T], F32R)             # K^T
    vt = persist.tile([H, T], F32R)             # V^T
    vsb = persist.tile([128, T // KT, H + 1], F32R)  # V natural tiles + ones col
    nc.vector.memset(vsb[:, :, H : H + 1].bitcast(F32), 1.0)

    for g in range(NG):
        sl = slice(GQ * g, GQ * (g + 1))
        # ---- projections for t-span sl ----
        qk_ps = ps_qk.tile([128, GQ], F32)
        v_ps = ps_v.tile([H, GQ], F32)
        for ci in range(CC):
            xt_t = xt_pool.tile([128, GQ], F32R)
            nc.sync.dma_start(out=xt_t[:], in_=xT[128 * ci : 128 * (ci + 1), sl])
            nc.tensor.matmul(qk_ps[:], wqk_sb[:, ci, :], xt_t[:],
                             start=(ci == 0), stop=(ci == CC - 1))
            nc.tensor.matmul(v_ps[:], wv_sb[:, ci, :], xt_t[:],
                             start=(ci == 0), stop=(ci == CC - 1))
        nc.scalar.copy(qt[:, sl], qk_ps[0:H, :])
        nc.scalar.copy(kt[:, sl], qk_ps[H:128, :])
        nc.vector.tensor_copy(vt[:, sl], v_ps[:, :])
        # ---- V^T -> natural V tiles (PE transpose, f32r 1.5 cyc/row) ----
        for jj in range(4):
            j = 4 * g + jj
            tr_ps = ps_tr.tile([KT, H], F32R)
            nc.tensor.transpose(tr_ps[:], vt[:, KT * j : KT * (j + 1)], ident[:])
            nc.scalar.copy(vsb[:, j, 0:H], tr_ps[:])
        # ---- attention for q-group g ----
        o_ps = ps_o.tile([H + 1, GQ], F32)
        jmax = 4 * g + 3
        for j in range(jmax + 1):
            s = j - 4 * g                       # diagonal sub-block index
            qlo = max(0, 128 * s)               # first valid q column
            s_ps = ps_s.tile([128, GQ], F32)
            nc.tensor.matmul(s_ps[:, qlo:GQ], kt[:, KT * j : KT * (j + 1)],
                             qt[:, GQ * g + qlo : GQ * (g + 1)],
                             start=True, stop=True)
            pt_t = pt_pool.tile([128, GQ], F32R)
            nc.scalar.activation(pt_t[:, qlo:GQ], s_ps[:, qlo:GQ], EXP)
            if s >= 0:
                nc.vector.tensor_mul(pt_t[:, qlo : qlo + 128],
                                     pt_t[:, qlo : qlo + 128], mask[:])
            nc.tensor.matmul(o_ps[:, qlo:GQ], vsb[:, j, :], pt_t[:, qlo:GQ],
                             start=(j == 0), stop=(j == jmax))
        # ---- normalize and store ----
        rec = div_pool.tile([1, GQ], F32)
        nc.vector.reciprocal(rec[:], o_ps[H : H + 1, :])
        dbc = div_pool.tile([H, GQ], F32)
        nc.gpsimd.partition_broadcast(dbc[:], rec[:])
        osb = out_pool.tile([H, GQ], F32)
        nc.vector.tensor_mul(osb[:], o_ps[0:H, :], dbc[:])
        nc.sync.dma_start(out=outT[:, sl], in_=osb[:])


def build():
    nc = bacc.Bacc("TRN2", target_bir_lowering=False, debug=False)
    with tile.TileContext(nc) as tc:
        with ExitStack() as ctx:
            _emit(ctx, tc)
    nc.compile()
    return nc


_NC_CACHE = None


def _get_module():
    global _NC_CACHE
    if _NC_CACHE is None:
        _NC_CACHE = build()
    return _NC_CACHE


def prep_in_maps(x, Wq, Wk, Wv):
    x = np.asarray(x, dtype=np.float32)
    Wq = np.asarray(Wq, dtype=np.float32)
    Wk = np.asarray(Wk, dtype=np.float32)
    Wv = np.asarray(Wv, dtype=np.float32)
    wqk = np.ascontiguousarray(
        np.concatenate([Wq * (1.0 / np.sqrt(H)), Wk], axis=1), dtype=np.float32)
    wv = np.ascontiguousarray(Wv)
    return [
        {"xT": np.ascontiguousarray(x[b].T), "wqk": wqk, "wv": wv}
        for b in range(B)
    ]


def assemble_out(results):
    out = np.empty((B, T, H), dtype=np.float32)
    for b in range(B):
        out[b] = results[b]["outT"].T
    return out


def run(x, Wq, Wk, Wv, trace=False):
    nc = _get_module()
    in_maps = prep_in_maps(x, Wq, Wk, Wv)
    res = run_bass_kernel_spmd(nc, in_maps, core_ids=list(range(N_CORES)),
                               trace=trace)
    return assemble_out(res.results), res


def kernel(x, Wq, Wk, Wv):
    out, _ = run(x, Wq, Wk, Wv)
    return out



# revision 6
# speedup vs baseline: 1.4750x; 1.4750x over previous
"""Single-head causal attention (B=8, T=2048, C=1024, H=64) on 8 TRN2 NeuronCores.

Sharding: data-parallel over batch -- core b computes batch element b. No
collectives. Per core, for x_b [T, C]:
    q = x_b @ Wq / sqrt(H); k = x_b @ Wk; v = x_b @ Wv
    out = softmax(causal(q @ k.T)) @ v

v2 layout strategy (all PE work in bf16, 1 cycle/row + FWL weight loads):
  - Host passes xT = x_b.T [C, T] pre-converted to bf16 (halves input DMA)
    and wqkv packed [128, 8, 192] bf16 (Wq/sqrt(H) | Wk | Wv per C-chunk) so
    weights land in one contiguous DMA.
  - Projections per 512-wide t-group: accumulate QK^T [128, 512] and V^T
    [64, 512] in PSUM over 8 C-chunks; copy out as bf16 Q^T/K^T [64, T] and
    re-tile V^T pairwise into vt2 [128, 8, 128] so ONE PE transpose per pair
    of 128-k-tiles yields natural V [128k, 2x64h] tiles. V tiles live padded
    in vsb [128, 16, 128]: col 64 = ones (softmax denominators fall out of
    the O matmul), cols 65:128 = zero pad so the stationary is 128-wide
    (enables fast weight load).
  - Attention per 512-wide q-group g, k-tiles processed in PAIRS: two full
    width scores matmuls S^T = K_j^T.T @ Q^T into a 2-bank PSUM tile, ONE
    exp over [128, 1024] on ACT (no max subtraction; scores bounded ~+-7),
    causal 0/1 mask multiply on the single diagonal 128x128 sub-block, then
    per-tile O^T[128, qlo:512] += Vpad_j.T @ P^T accumulated in PSUM. Row 64
    of O^T is the softmax denominator.
  - Normalize: DVE reciprocal_approx_fast on the denominator row, GPSIMD
    partition broadcast, DVE multiply; DMA O^T [64, T] f32 out. Host
    transposes back.
"""

from contextlib import ExitStack

import ml_dtypes
import numpy as np

import concourse.mybir as mybir
import concourse.tile as tile
from concourse import bacc
from concourse.bass_utils import run_bass_kernel_spmd
from concourse.masks import make_identity, make_upper_triangular

B, T, C, H = 8, 2048, 1024, 64
N_CORES = 8
GQ = 512          # q-group width (PSUM bank)
NG = T // GQ      # 4 q-groups
KT = 128          # k-tile size
CC = C // 128     # 8 contraction chunks
F32 = mybir.dt.float32
BF = mybir.dt.bfloat16
EXP = mybir.ActivationFunctionType.Exp


def _emit(ctx, tc):
    nc = tc.nc
    xT = nc.dram_tensor("xT", [C, T], BF, kind="ExternalInput").ap()
    wqkv = nc.dram_tensor("wqkv", [128, CC, 3 * H], BF, kind="ExternalInput").ap()
    outT = nc.dram_tensor("outT", [H, T], F32, kind="ExternalOutput").ap()

    const = ctx.enter_context(tc.tile_pool(name="const", bufs=1))
    persist = ctx.enter_context(tc.tile_pool(name="persist", bufs=1))
    xt_pool = ctx.enter_context(tc.tile_pool(name="xt", bufs=10))
    pt_pool = ctx.enter_context(tc.tile_pool(name="pt", bufs=3))
    out_pool = ctx.enter_context(tc.tile_pool(name="outp", bufs=2))
    div_pool = ctx.enter_context(tc.tile_pool(name="div", bufs=2))
    ps_s = ctx.enter_context(tc.tile_pool(name="ps_s", bufs=2, space="PSUM"))
    ps_o = ctx.enter_context(tc.tile_pool(name="ps_o", bufs=2, space="PSUM"))
    ps_qk = ctx.enter_context(tc.tile_pool(name="ps_qk", bufs=1, space="PSUM"))
    ps_vtr = ctx.enter_context(tc.tile_pool(name="ps_vtr", bufs=1, space="PSUM"))

    wsb = const.tile([128, CC, 3 * H], BF)
    nc.sync.dma_start(out=wsb[:], in_=wqkv)
    # mask[p, f] = 1.0 where p <= f else 0 : keep k_local <= q_local.
    maskf = const.tile([128, 128], F32)
    make_upper_triangular(nc, maskf[:], val=1.0, diag=True)
    mask = const.tile([128, 128], BF)
    nc.scalar.copy(mask[:], maskf[:])
    identf = const.tile([128, 128], F32)
    make_identity(nc, identf[:])
    ident = const.tile([128, 128], BF)
    nc.scalar.copy(ident[:], identf[:])

    qt = persist.tile([H, T], BF)               # Q^T (pre-scaled by 1/sqrt(H))
    kt = persist.tile([H, T], BF)               # K^T
    vt2 = persist.tile([128, T // (2 * KT), 128], BF)  # V^T pair-packed
    vsb = persist.tile([128, T // KT, 128], BF)  # natural V tiles, padded
    nc.vector.memset(vsb[:, :, H : H + 1], 1.0)
    nc.vector.memset(vsb[:, :, H + 1 : 128], 0.0)

    for g in range(NG):
        sl = slice(GQ * g, GQ * (g + 1))
        # ---- projections for t-span sl ----
        qk_ps = ps_qk.tile([128, GQ], F32)
        vtr = ps_vtr.tile([128, GQ], F32)
        for ci in range(CC):
            xt_t = xt_pool.tile([128, GQ], BF)
            nc.sync.dma_start(out=xt_t[:], in_=xT[128 * ci : 128 * (ci + 1), sl])
            nc.tensor.matmul(qk_ps[:], wsb[:, ci, 0:128], xt_t[:],
                             start=(ci == 0), stop=(ci == CC - 1))
            nc.tensor.matmul(vtr[0:H, :], wsb[:, ci, 128:192], xt_t[:],
                             start=(ci == 0), stop=(ci == CC - 1))
        nc.vector.tensor_copy(qt[:, sl], qk_ps[0:H, :])
        nc.scalar.copy(kt[:, sl], qk_ps[H:128, :])
        # ---- V^T -> pair-packed vt2, then PE-transpose to natural V ----
        for e in range(2):
            u = 2 * g + e
            nc.vector.tensor_copy(vt2[0:H, u, :], vtr[0:H, 256 * e : 256 * e + 128])
            # partition-shifted copy (0:64 -> 64:128) must run on ACT
            nc.scalar.copy(vt2[H:128, u, :],
                           vtr[0:H, 256 * e + 128 : 256 * e + 256])
        vtr_b = vtr.bitcast(BF)                 # reuse the V PSUM bank
        for e in range(2):
            u = 2 * g + e
            nc.tensor.transpose(vtr_b[:, 128 * e : 128 * (e + 1)],
                                vt2[:, u, :], ident[:])
            nc.vector.tensor_copy(vsb[:, 2 * u, 0:H],
                                  vtr_b[:, 128 * e : 128 * e + H])
            nc.vector.tensor_copy(vsb[:, 2 * u + 1, 0:H],
                                  vtr_b[:, 128 * e + H : 128 * e + 128])
        # ---- attention for q-group g: k-tile pairs ----
        o_ps = ps_o.tile([128, GQ], F32)
        jmax = 4 * g + 3
        for j0 in range(0, jmax + 1, 2):
            s_ps = ps_s.tile([128, 2, GQ], F32)
            for e in range(2):
                j = j0 + e
                nc.tensor.matmul(s_ps[:, e, :], kt[:, KT * j : KT * (j + 1)],
                                 qt[:, sl], start=True, stop=True)
            pt_t = pt_pool.tile([128, 2, GQ], BF)
            nc.scalar.activation(pt_t[:], s_ps[:], EXP)
            for e in range(2):
                j = j0 + e
                s = j - 4 * g                   # diagonal sub-block index
                if s >= 0:
                    qlo = 128 * s
                    nc.vector.tensor_mul(pt_t[:, e, qlo : qlo + 128],
                                         pt_t[:, e, qlo : qlo + 128], mask[:])
            for e in range(2):
                j = j0 + e
                qlo = max(0, 128 * (j - 4 * g))
                nc.tensor.matmul(o_ps[:, qlo:GQ], vsb[:, j, :],
                                 pt_t[:, e, qlo:GQ],
                                 start=(j == 0), stop=(j == jmax))
        # ---- normalize and store ----
        dn = div_pool.tile([1, GQ], F32)
        nc.scalar.copy(dn[:], o_ps[H : H + 1, :])
        rec = div_pool.tile([1, GQ], F32)
        nc.vector.reciprocal_approx_fast(rec[:], dn[:])
        dbc = div_pool.tile([H, GQ], F32)
        nc.gpsimd.partition_broadcast(dbc[:], rec[:])
        osb = out_pool.tile([H, GQ], F32)
        nc.vector.tensor_mul(osb[:], o_ps[0:H, :], dbc[:])
        nc.sync.dma_start(out=outT[:, sl], in_=osb[:])


def build():
    nc = bacc.Bacc("TRN2", target_bir_lowering=False, debug=False)
    with tile.TileContext(nc) as tc:
        with ExitStack() as ctx:
            _emit(ctx, tc)
    nc.compile()
    return nc


_NC_CACHE = None


def _get_module():
    global _NC_CACHE
    if _NC_CACHE is None:
        _NC_CACHE = build()
    return _NC_CACHE


def prep_in_maps(x, Wq, Wk, Wv):
    x = np.asarray(x, dtype=np.float32)
    Wq = np.asarray(Wq, dtype=np.float32)
    Wk = np.asarray(Wk, dtype=np.float32)
    Wv = np.asarray(Wv, dtype=np.float32)
    bf16 = ml_dtypes.bfloat16
    # [C, 192] = [Wq/sqrt(H) | Wk | Wv], tiled to [128, CC, 192]
    wcat = np.concatenate([Wq * (1.0 / np.sqrt(H)), Wk, Wv], axis=1)
    wqkv = np.ascontiguousarray(
        wcat.reshape(CC, 128, 3 * H).transpose(1, 0, 2).astype(bf16))
    return [
        {"xT": np.ascontiguousarray(x[b].T.astype(bf16)), "wqkv": wqkv}
        for b in range(B)
    ]


def assemble_out(results):
    out = np.empty((B, T, H), dtype=np.float32)
    for b in range(B):
        out[b] = results[b]["outT"].T
    return out


def run(x, Wq, Wk, Wv, trace=False):
    nc = _get_module()
    in_maps = prep_in_maps(x, Wq, Wk, Wv)
    res = run_bass_kernel_spmd(nc, in_maps, core_ids=list(range(N_CORES)),
                               trace=trace)
    return assemble_out(res.results), res


def kernel(x, Wq, Wk, Wv):
    out, _ = run(x, Wq, Wk, Wv)
    return out
